# revision 19
# baseline (speedup 1.0000x reference)
"""Trainium2 Bass kernel for per-element tiny MLPs.

Problem: N=4,000,000 independent 1->8->1 MLPs:
    y[i] = W2[i] @ relu(W1[i] * x[i] + b1[i]) + b2[i]

Memory-bound: 104 B/net in + 4 B/net out. Sharded over 8 NeuronCores by net
index (data parallel, no communication).

Device layout (per core, R=500,000 nets padded to R_PAD=128*3907): natural
interleaved layout — tile t covers 128*Fi nets; partition p holds nets
[base + p*Fi, base + (p+1)*Fi); the hidden dim j stays innermost in the free
dim, i.e. a weight tile is [128, Fi*8] and is a contiguous slice of the
natural [N, 8] array. No host-side repacking beyond pad+slice.

Per tile (everything in the free dim; no PE, no PSUM):
  DVE    : z = broadcast(x) * W1    (in0 has a step-0 inner AP dim - exact)
  DVE    : z = z + b1
  ACT    : z = relu(z)
  GPSIMD : u = z * W2
  DVE    : y = segmented_reduce_8(u)   (tensor_reduce axis=X on [128,Fi,8])
  DVE    : y = y + b2
"""

import numpy as np
from contextlib import ExitStack

import concourse.bacc as bacc
import concourse.mybir as mybir
import concourse.tile as tile
from concourse.bass_utils import run_bass_kernel_spmd

F32 = mybir.dt.float32
AF = mybir.ActivationFunctionType
OP = mybir.AluOpType
AX = mybir.AxisListType

N = 4_000_000
H = 8
N_CORES = 8
R = N // N_CORES            # 500,000 nets per core
FP = 3907                   # nets per partition (padded): 128*3907 = 500,096
R_PAD = 128 * FP
FIS = [288] * 13 + [163]    # per-tile nets-per-partition; sum == FP


def build_nc(fis):
    fp = sum(fis)
    rp = 128 * fp

    nc = bacc.Bacc("TRN2", target_bir_lowering=False, debug=False)

    w1 = nc.dram_tensor("w1", [rp, H], F32, kind="ExternalInput")
    b1 = nc.dram_tensor("b1", [rp, H], F32, kind="ExternalInput")
    w2 = nc.dram_tensor("w2", [rp, H], F32, kind="ExternalInput")
    xs = nc.dram_tensor("xs", [rp], F32, kind="ExternalInput")
    b2 = nc.dram_tensor("b2", [rp], F32, kind="ExternalInput")
    ys = nc.dram_tensor("ys", [rp], F32, kind="ExternalOutput")

    with tile.TileContext(nc) as tc, ExitStack() as ctx:
        wpool = ctx.enter_context(tc.tile_pool(name="w", bufs=3))
        zpool = ctx.enter_context(tc.tile_pool(name="z", bufs=2))
        vpool = ctx.enter_context(tc.tile_pool(name="v", bufs=4))

        nb = 0
        for fi in fis:
            nrows = 128 * fi
            wsl = lambda t: t.ap()[nb:nb + nrows, :].rearrange(
                "(p f) j -> p (f j)", p=128
            )
            vsl = lambda t: t.ap()[nb:nb + nrows].rearrange("(p f) -> p f", p=128)

            w1t = wpool.tile([128, fi * H], F32, tag="w1t")
            nc.sync.dma_start(w1t[:], wsl(w1))
            b1t = wpool.tile([128, fi * H], F32, tag="b1t")
            nc.scalar.dma_start(b1t[:], wsl(b1))
            w2t = wpool.tile([128, fi * H], F32, tag="w2t")
            nc.sync.dma_start(w2t[:], wsl(w2))
            xt = vpool.tile([128, fi], F32, tag="xt")
            nc.scalar.dma_start(xt[:], vsl(xs))
            b2t = vpool.tile([128, fi], F32, tag="b2t")
            nc.scalar.dma_start(b2t[:], vsl(b2))

            w1t3 = w1t[:].rearrange("p (f j) -> p f j", j=H)
            xb = xt[:].broadcast_to([128, fi, H])

            za = zpool.tile([128, fi * H], F32, tag="za")
            zb = zpool.tile([128, fi * H], F32, tag="zb")
            zc = zpool.tile([128, fi * H], F32, tag="zc")
            zd = zpool.tile([128, fi * H], F32, tag="zd")

            nc.vector.tensor_tensor(
                za[:].rearrange("p (f j) -> p f j", j=H), xb, w1t3, op=OP.mult
            )
            nc.vector.tensor_tensor(zb[:], za[:], b1t[:], op=OP.add)
            nc.scalar.activation(zc[:], zb[:], AF.Relu)
            nc.vector.tensor_tensor(zd[:], zc[:], w2t[:], op=OP.mult)

            yt = vpool.tile([128, fi], F32, tag="yt")
            nc.vector.tensor_reduce(
                yt[:], zd[:].rearrange("p (f j) -> p f j", j=H), axis=AX.X, op=OP.add
            )
            yo = vpool.tile([128, fi], F32, tag="yo")
            nc.vector.tensor_tensor(yo[:], yt[:], b2t[:], op=OP.add)

            nc.sync.dma_start(vsl(ys), yo[:])
            nb += nrows

    nc.compile()
    return nc


# ---------------- entry point ----------------

_CACHE = {}


def _get_nc():
    if "nc" not in _CACHE:
        _CACHE["nc"] = build_nc(FIS)
    return _CACHE["nc"]


def _pad2(a):
    out = np.zeros((R_PAD, H), np.float32)
    out[:R] = a
    return out


def _pad1(a):
    out = np.zeros(R_PAD, np.float32)
    out[:R] = a
    return out


def _make_in_maps(x, W1, b1, W2, b2):
    x = np.ascontiguousarray(x, np.float32)
    b2 = np.ascontiguousarray(b2, np.float32)
    in_maps = []
    for c in range(N_CORES):
        sl = slice(c * R, (c + 1) * R)
        in_maps.append({
            "w1": _pad2(np.asarray(W1[sl], np.float32)),
            "b1": _pad2(np.asarray(b1[sl], np.float32)),
            "w2": _pad2(np.asarray(W2[sl], np.float32)),
            "xs": _pad1(x[sl, 0]),
            "b2": _pad1(b2[sl, 0]),
        })
    return in_maps


def _run(x, W1, b1, W2, b2, **kw):
    nc = _get_nc()
    res = run_bass_kernel_spmd(nc, _make_in_maps(x, W1, b1, W2, b2),
                               core_ids=list(range(N_CORES)), **kw)
    y = np.empty((N, 1), np.float32)
    for c in range(N_CORES):
        y[c * R:(c + 1) * R, 0] = res.results[c]["ys"].reshape(-1)[:R]
    return y, res


def kernel(x, W1, b1, W2, b2):
    y, _ = _run(x, W1, b1, W2, b2)
    return y


# revision 21
# speedup vs baseline: 1.1086x; 1.1086x over previous
"""Trainium2 Bass kernel for per-element tiny MLPs.

Problem: N=4,000,000 independent 1->8->1 MLPs:
    y[i] = W2[i] @ relu(W1[i] * x[i] + b1[i]) + b2[i]

Memory-bound: 104 B/net in + 4 B/net out. Sharded over 8 NeuronCores by net
index (data parallel, no communication).

Device layout (per core, R=500,000 nets padded to R_PAD=128*3907): natural
interleaved layout — tile t covers 128*Fi nets; partition p holds nets
[base + p*Fi, base + (p+1)*Fi); the hidden dim j stays innermost in the free
dim, i.e. a weight tile is [128, Fi*8] and is a contiguous slice of the
natural [N, 8] array. No host-side repacking beyond pad+slice.

Per tile (everything in the free dim; no PE, no PSUM):
  DVE    : z = broadcast(x) * W1    (in0 has a step-0 inner AP dim - exact)
  DVE    : z = z + b1
  ACT    : z = relu(z)
  GPSIMD : u = z * W2
  DVE    : y = segmented_reduce_8(u)   (tensor_reduce axis=X on [128,Fi,8])
  DVE    : y = y + b2
"""

import numpy as np
from contextlib import ExitStack

import concourse.bacc as bacc
import concourse.mybir as mybir
import concourse.tile as tile
from concourse.bass_utils import run_bass_kernel_spmd

F32 = mybir.dt.float32
AF = mybir.ActivationFunctionType
OP = mybir.AluOpType
AX = mybir.AxisListType

N = 4_000_000
H = 8
N_CORES = 8
R = N // N_CORES            # 500,000 nets per core
FP = 3907                   # nets per partition (padded): 128*3907 = 500,096
R_PAD = 128 * FP
FIS = [288] * 13 + [163]    # per-tile nets-per-partition; sum == FP


def build_nc(fis):
    fp = sum(fis)
    rp = 128 * fp

    nc = bacc.Bacc("TRN2", target_bir_lowering=False, debug=False)

    w1 = nc.dram_tensor("w1", [rp, H], F32, kind="ExternalInput")
    b1 = nc.dram_tensor("b1", [rp, H], F32, kind="ExternalInput")
    w2 = nc.dram_tensor("w2", [rp, H], F32, kind="ExternalInput")
    xs = nc.dram_tensor("xs", [rp], F32, kind="ExternalInput")
    b2 = nc.dram_tensor("b2", [rp], F32, kind="ExternalInput")
    ys = nc.dram_tensor("ys", [rp], F32, kind="ExternalOutput")

    with tile.TileContext(nc) as tc, ExitStack() as ctx:
        wpool = ctx.enter_context(tc.tile_pool(name="w", bufs=3))
        zpool = ctx.enter_context(tc.tile_pool(name="z", bufs=2))
        vpool = ctx.enter_context(tc.tile_pool(name="v", bufs=4))

        nb = 0
        for fi in fis:
            nrows = 128 * fi
            wsl = lambda t: t.ap()[nb:nb + nrows, :].rearrange(
                "(p f) j -> p (f j)", p=128
            )
            vsl = lambda t: t.ap()[nb:nb + nrows].rearrange("(p f) -> p f", p=128)

            w1t = wpool.tile([128, fi * H], F32, tag="w1t")
            nc.sync.dma_start(w1t[:], wsl(w1))
            b1t = wpool.tile([128, fi * H], F32, tag="b1t")
            nc.scalar.dma_start(b1t[:], wsl(b1))
            w2t = wpool.tile([128, fi * H], F32, tag="w2t")
            nc.sync.dma_start(w2t[:], wsl(w2))
            xt = vpool.tile([128, fi], F32, tag="xt")
            nc.scalar.dma_start(xt[:], vsl(xs))
            b2t = vpool.tile([128, fi], F32, tag="b2t")
            nc.scalar.dma_start(b2t[:], vsl(b2))

            w1t3 = w1t[:].rearrange("p (f j) -> p f j", j=H)
            xb = xt[:].broadcast_to([128, fi, H])

            za = zpool.tile([128, fi * H], F32, tag="za")
            zb = zpool.tile([128, fi * H], F32, tag="zb")
            zc = zpool.tile([128, fi * H], F32, tag="zc")
            zd = zpool.tile([128, fi * H], F32, tag="zd")

            nc.vector.tensor_tensor(
                za[:].rearrange("p (f j) -> p f j", j=H), xb, w1t3, op=OP.mult
            )
            nc.vector.tensor_tensor(zb[:], za[:], b1t[:], op=OP.add)
            nc.scalar.activation(zc[:], zb[:], AF.Relu)
            nc.vector.tensor_tensor(zd[:], zc[:], w2t[:], op=OP.mult)

            yt = vpool.tile([128, fi], F32, tag="yt")
            nc.vector.tensor_reduce(
                yt[:], zd[:].rearrange("p (f j) -> p f j", j=H), axis=AX.X, op=OP.add
            )
            yo = vpool.tile([128, fi], F32, tag="yo")
            nc.vector.tensor_tensor(yo[:], yt[:], b2t[:], op=OP.add)

            nc.sync.dma_start(vsl(ys), yo[:])
            nb += nrows

    nc.compile()
    return nc


# ---------------- entry point ----------------

_CACHE = {}


def _get_nc():
    if "nc" not in _CACHE:
        _CACHE["nc"] = build_nc(FIS)
    return _CACHE["nc"]


def _pad2(a):
    out = np.zeros((R_PAD, H), np.float32)
    out[:R] = a
    return out


def _pad1(a):
    out = np.zeros(R_PAD, np.float32)
    out[:R] = a
    return out


def _make_in_maps(x, W1, b1, W2, b2):
    x = np.ascontiguousarray(x, np.float32)
    b2 = np.ascontiguousarray(b2, np.float32)
    in_maps = []
    for c in range(N_CORES):
        sl = slice(c * R, (c + 1) * R)
        in_maps.append({
            "w1": _pad2(np.asarray(W1[sl], np.float32)),
            "b1": _pad2(np.asarray(b1[sl], np.float32)),
            "w2": _pad2(np.asarray(W2[sl], np.float32)),
            "xs": _pad1(x[sl, 0]),
            "b2": _pad1(b2[sl, 0]),
        })
    return in_maps


def _run(x, W1, b1, W2, b2, **kw):
    nc = _get_nc()
    res = run_bass_kernel_spmd(nc, _make_in_maps(x, W1, b1, W2, b2),
                               core_ids=list(range(N_CORES)), **kw)
    y = np.empty((N, 1), np.float32)
    for c in range(N_CORES):
        y[c * R:(c + 1) * R, 0] = res.results[c]["ys"].reshape(-1)[:R]
    return y, res


def kernel(x, W1, b1, W2, b2):
    y, _ = _run(x, W1, b1, W2, b2)
    return y
